# revision 11
# baseline (speedup 1.0000x reference)
"""Pointer-style attention kernel for Trainium2, SPMD over 8 NeuronCores.

Reference computation (per full batch B=128, S=2048, E=H=512):
    q  = query @ Wq.T + bq                    [B, H]
    k  = target @ Wk.T + bk                   [B, S, H]
    qk = einsum('bh,bsh->bs', q, k)           [B, S]
    qk = 10 * tanh(qk);  qk[mask==1] = -inf
    alpha = softmax(qk, axis=-1)

Key algebraic reformulation (exact in exact arithmetic):
    qk[b,s] = target[b,s,:] . qp[b,:] + qb[b]
      qp = (query @ Wq.T + bq) @ Wk           [B, E]
      qb = (query @ Wq.T + bq) . bk           [B]
This collapses the S*E*H einsum (137 GFLOP) into an S*E dot-product
stream (0.27 GFLOP), making the kernel HBM-bound on streaming `target`
(64 MiB per core; ~187 us floor at the 358 GB/s per-core HBM limit).

Distribution: data-parallel over batch; 16 batches per core, weights
replicated, no cross-core communication (softmax is per-row).

Per-core plan (v2):
  - target is streamed as 32 half-batch units of [128, 8, 512] fp32,
    alternating between the two HWDGE rings (sync/SP and scalar/ACT)
    so per-DMA completion latency on one ring hides under the other's
    stream. The s<->(partition,row) mapping s = 1024h + 8p + j makes
    each partition's 16 KB contiguous in HBM (fat descriptors); the
    resulting output permutation is undone on the host.
  - a single fused DVE tensor_tensor_reduce per (batch, s-row) does
    mul+reduce+bias in one pass: scores = qb + sum_e target*qp, with
    the product written to a stride-0 dummy (never materialized).
    ~4.5 us/unit on Vector vs 5.86 us/unit of DMA -> compute never
    backpressures the stream.
  - TensorE builds qp per batch as a broadcast matmul into PSUM
    (consumed directly by the DVE as in1) and handles all transposes;
    epilogue (tanh/exp/mask/softmax-normalize) runs per 2 batches in
    engine-idle slots, so the post-stream tail is only the last pair.
"""

import sys
import types

import numpy as np

B, S, E, H = 128, 2048, 512, 512
C_CLIP = 10.0
NCORES = 8
BS = B // NCORES  # 16 batches per core
EC = E // 128  # 4 e/h-chunks of 128
HK = 8  # s-rows per partition per unit; s = 1024h + 8p + j
CPB = 16  # score columns per batch (c = 8h + j)
NU = BS * 2  # 32 half-batch pipeline units


def _install_axon_profile_shim():
    """Make run_bass_kernel_spmd(trace=True) usable in this container:
    provide antenv.axon_hooks (NTFF profile hook via ctypes into the
    axon PJRT .so) and stub the S3 artifact upload."""
    try:
        if "antenv.axon_hooks" not in sys.modules:
            import antenv
            from trn_agent_boot.trn_boot import _ntff_profile_via_ctypes

            hook = _ntff_profile_via_ctypes("/opt/axon/libaxon_pjrt.so")
            mod = types.ModuleType("antenv.axon_hooks")
            mod._hook = hook
            mod.get_axon_ntff_profile_hook = lambda: mod._hook

            def _set(h):
                mod._hook = h

            mod.set_axon_ntff_profile_hook = _set
            sys.modules["antenv.axon_hooks"] = mod
            antenv.axon_hooks = mod
    except Exception:
        pass
    try:
        import concourse.bass_utils as bu

        bu.upload_artifacts = lambda tmpdir: str(tmpdir)
    except Exception:
        pass


def _legalize_sync_waits(nc):
    """This walrus build rejects instructions carrying more than a couple
    of sync-wait commands. After Tile scheduling, split each instruction's
    excess waits onto same-engine NOPs inserted immediately before it —
    sequencers execute in order, so semantics are identical."""
    import bass_rust
    from concourse import mybir

    n_split = 0
    for f in nc.m.functions:
        for blk in f.blocks:
            il = blk.instructions
            out = []
            changed = False
            for inst in il:
                si = inst.sync_info
                waits = list(si.on_wait) if si is not None else []
                cap = 2 if isinstance(inst, mybir.InstEventSemaphore) else 1
                if len(waits) > cap:
                    rest = waits[: len(waits) - cap]
                    for j, w in enumerate(rest):
                        nop = mybir.InstNoOp(
                            name=f"{inst.name}-swait{j}",
                            engine=inst.engine,
                            bass_nofuse=True,
                            sync_info=bass_rust.SyncInfo(on_wait=[w], on_update=[]),
                        )
                        out.append(nop)
                        n_split += 1
                    si.on_wait = waits[len(waits) - cap :]
                    inst.sync_info = si
                    changed = True
                out.append(inst)
            if changed:
                blk.instructions = out
    return n_split


def build_kernel():
    import concourse.bass as bass
    import concourse.tile as tile
    from concourse import mybir
    from concourse.masks import make_identity

    f32 = mybir.dt.float32
    bf16 = mybir.dt.bfloat16
    Alu = mybir.AluOpType
    Act = mybir.ActivationFunctionType

    nc = bass.Bass()
    # host passes layout-transformed views: queryT/WqT transposed, biases
    # in [p, chunk] column form, mask as a permuted keep-multiplier m01P
    queryT_d = nc.dram_tensor("queryT", [E, BS], f32, kind="ExternalInput")
    target_d = nc.dram_tensor("target", [BS, S, E], f32, kind="ExternalInput")
    m01P_d = nc.dram_tensor("m01P", [128, BS * CPB], f32, kind="ExternalInput")
    wqT_d = nc.dram_tensor("WqT", [E, H], f32, kind="ExternalInput")
    wk_d = nc.dram_tensor("Wk", [H, E], f32, kind="ExternalInput")
    bqT_d = nc.dram_tensor("bqT", [128, EC], f32, kind="ExternalInput")
    bkT_d = nc.dram_tensor("bkT", [128, EC], f32, kind="ExternalInput")
    alphaP_d = nc.dram_tensor("alphaP", [BS * CPB, 128], f32, kind="ExternalOutput")

    # unit (b, h): partition p holds s-rows 1024h + 8p + j, j=0..7 —
    # 16 KB contiguous per partition per unit
    units = target_d.rearrange("b (h p k) e -> (b h) p k e", h=2, p=128, k=HK)

    with tile.TileContext(nc) as tc:
        with (
            tc.tile_pool(name="singles", bufs=1) as singles,
            tc.tile_pool(name="tgt", bufs=8) as tgtp,
            tc.tile_pool(name="prod", bufs=2) as prodp,
            tc.tile_pool(name="trash", bufs=2) as trashp,
            tc.tile_pool(name="epi", bufs=2) as epip,
            tc.tile_pool(name="ppre", bufs=2, space="PSUM") as ppre,
            tc.tile_pool(name="pqpb", bufs=3, space="PSUM") as pqpb,
            tc.tile_pool(name="pepi", bufs=2, space="PSUM") as pepi,
        ):
            # ---- weight/query/mask DMAs: scalar(ACT) ring; wk on sync(SP)
            # ring ahead of the even target units. Target units alternate
            # rings, so both rings stream back-to-back from t=0.
            qT_sb = singles.tile([128, EC, BS], f32)  # queryT [e'-part, b]
            nc.scalar.dma_start(
                out=qT_sb, in_=queryT_d.rearrange("(m p) b -> p m b", p=128)
            )
            wqT_sb = singles.tile([128, EC, H], f32)  # [p, e'-chunk, h]
            for c in range(EC):
                nc.scalar.dma_start(
                    out=wqT_sb[:, c, :], in_=wqT_d[c * 128 : (c + 1) * 128, :]
                )
            bqT = singles.tile([128, EC], f32)
            bkT = singles.tile([128, EC], f32)
            nc.scalar.dma_start(out=bqT, in_=bqT_d[:, :])
            nc.scalar.dma_start(out=bkT, in_=bkT_d[:, :])
            wk_sb = singles.tile([128, EC, E], f32)  # [p, h-chunk, e]
            for c in range(EC):
                nc.sync.dma_start(
                    out=wk_sb[:, c, :], in_=wk_d[c * 128 : (c + 1) * 128, :]
                )
            m01T = singles.tile([128, BS, CPB], f32)  # keep-multiplier
            nc.scalar.dma_start(
                out=m01T, in_=m01P_d.rearrange("p (b c) -> p b c", b=BS)
            )

            ident = singles.tile([128, 128], f32)
            make_identity(nc, ident)
            ones_row = singles.tile([1, 128], f32)  # lhsT for partition-bcast
            nc.vector.memset(ones_row, 1.0)
            ones_col = singles.tile([128, 1], f32)  # lhsT for partition-sum
            nc.vector.memset(ones_col, 1.0)

            # Warm the PE clock gate (HAM) while the weight DMAs land.
            for _ in range(16):
                pwrm = ppre.tile([128, 128], f32, tag="pre")
                nc.tensor.matmul(pwrm, ident, ident, start=True, stop=True)

            # q = query @ Wq.T  [BS, H]
            q_sb = singles.tile([BS, H], f32)
            pq2 = ppre.tile([BS, H], f32, tag="pre")
            for m in range(EC):
                nc.tensor.matmul(
                    pq2, qT_sb[:, m, :], wqT_sb[:, m, :],
                    start=(m == 0), stop=(m == EC - 1),
                )
            nc.scalar.copy(q_sb, pq2)
            # qhT = (q + bq) transposed to [h-part, b]
            qhT = singles.tile([128, EC, BS], f32)
            for c in range(EC):
                pq3 = ppre.tile([128, BS], f32, tag="pre")
                nc.tensor.transpose(
                    pq3, q_sb[:, c * 128 : (c + 1) * 128], ident[0:BS, 0:BS]
                )
                nc.scalar.copy(qhT[:, c, :], pq3)
                nc.vector.tensor_scalar(
                    out=qhT[:, c, :], in0=qhT[:, c, :],
                    scalar1=bqT[:, c : c + 1], scalar2=None, op0=Alu.add,
                )

            # qb[b] = (q + bq) . bk, broadcast to qbb [128, BS] — feeds the
            # fused reduce as its per-partition init value.
            pqb = ppre.tile([BS, 1], f32, tag="pre")
            for c in range(EC):
                nc.tensor.matmul(
                    pqb, qhT[:, c, :], bkT[:, c : c + 1],
                    start=(c == 0), stop=(c == EC - 1),
                )
            qb_sb = singles.tile([BS, 1], f32)
            nc.scalar.copy(qb_sb, pqb)
            pqbT = ppre.tile([1, BS], f32, tag="pre")
            nc.tensor.transpose(pqbT, qb_sb, ident[0:BS, 0:BS])
            qbrow = singles.tile([1, BS], f32)
            nc.scalar.copy(qbrow, pqbT)
            pqbb = ppre.tile([128, BS], f32, tag="pre")
            nc.tensor.matmul(pqbb, ones_row, qbrow, start=True, stop=True)
            qbb = singles.tile([128, BS], f32)
            nc.scalar.copy(qbb, pqbb)

            scores = singles.tile([128, BS, CPB], f32)
            e2 = singles.tile([128, BS, CPB], f32)
            a_sb = singles.tile([128, BS, CPB], f32)
            part = singles.tile([128, BS], f32)
            dummy = singles.tile([128, 1], f32)  # stride-0 sink for TTR out

            def _epi_pair(b0):
                """tanh/exp/mask/normalize batches b0, b0+1 and DMA them out.
                The qb[b] bias lands here, folded into the tanh's bias."""
                for b in (b0, b0 + 1):
                    t_t = epip.tile([128, CPB], f32, tag="tanh")
                    nc.scalar.activation(
                        t_t, scores[:, b, :], Act.Tanh,
                        bias=qbb[:, b : b + 1], scale=1.0,
                    )
                    nc.scalar.activation(e2[:, b, :], t_t, Act.Exp, scale=C_CLIP)
                for b in (b0, b0 + 1):
                    # fused: e2 *= m01 (mask) and part = row-sum, one pass
                    nc.vector.scalar_tensor_tensor(
                        out=e2[:, b, :], in0=e2[:, b, :], scalar=0.0,
                        in1=m01T[:, b, :], op0=Alu.bypass, op1=Alu.mult,
                        accum_out=part[:, b : b + 1],
                    )
                pden = pepi.tile([1, 2], f32, tag="epi")
                nc.tensor.matmul(
                    pden, ones_col, part[:, b0 : b0 + 2], start=True, stop=True
                )
                recip = epip.tile([1, 2], f32, tag="recip")
                nc.vector.reciprocal(recip, pden)
                prb = pepi.tile([128, 2], f32, tag="epi")
                nc.tensor.matmul(prb, ones_row, recip, start=True, stop=True)
                rb = epip.tile([128, 2], f32, tag="rb")
                nc.scalar.copy(rb, prb)
                for b in (b0, b0 + 1):
                    nc.vector.tensor_scalar(
                        out=a_sb[:, b, :], in0=e2[:, b, :],
                        scalar1=rb[:, b - b0 : b - b0 + 1], scalar2=None,
                        op0=Alu.mult,
                    )
                pat = pepi.tile([32, 128], f32, tag="epi")
                nc.tensor.transpose(pat, a_sb[:, b0 : b0 + 2, :], ident)
                at = epip.tile([32, 128], f32, tag="at")
                nc.scalar.copy(at, pat)
                nc.scalar.dma_start(
                    out=alphaP_d[b0 * CPB : (b0 + 2) * CPB, :], in_=at
                )

            # ---- main pipeline: stream target; per-unit reduction takes one
            # of two engine paths, balanced so both stay under the DMA rate:
            #   V path: fused scalar_tensor_tensor (V ~5.4 us/unit, S 0)
            #   S path: V multiply + ScalarE activation-accum (V ~4.2, S ~6.2)
            # The last two units go on the V path so the tail doesn't wait on
            # Scalar's longer chain.
            v_path = {0, 4, 8, 12, 16, 20, 24, 28, 30, 31}
            pb_cur = None
            for u in range(NU):
                b, h = divmod(u, 2)
                if h == 0:
                    # qp[b] broadcast across partitions, direct into PSUM
                    pb_cur = pqpb.tile([128, E], f32, tag="qpb")
                    for c in range(EC):
                        qrep = bass.AP(
                            tensor=qhT.tensor,
                            offset=qhT[:, c, b : b + 1].offset,
                            ap=[qhT.ap[0], [0, 128]],
                        )
                        nc.tensor.matmul(
                            pb_cur, qrep, wk_sb[:, c, :],
                            start=(c == 0), stop=(c == EC - 1),
                        )
                tgt = tgtp.tile([128, HK, E], f32, tag="tgt")
                eng = nc.sync if (u % 2 == 0) else nc.scalar
                eng.dma_start(out=tgt, in_=units[u])
                if u in v_path:
                    for j in range(HK):
                        nc.vector.scalar_tensor_tensor(
                            out=dummy.broadcast_to((128, E)),
                            in0=tgt[:, j, :],
                            scalar=0.0,
                            in1=pb_cur,
                            op0=Alu.bypass,
                            op1=Alu.mult,
                            accum_out=scores[:, b, h * HK + j : h * HK + j + 1],
                        )
                else:
                    prod = prodp.tile([128, HK, E], f32, tag="prod")
                    pb_b = bass.AP(
                        tensor=pb_cur.tensor, offset=pb_cur.offset,
                        ap=[pb_cur.ap[0], [0, HK], pb_cur.ap[1]],
                    )
                    nc.vector.tensor_mul(prod, tgt, pb_b)
                    for j in range(HK):
                        tr = trashp.tile([128, E], bf16, tag="tr")
                        nc.scalar.activation(
                            tr, prod[:, j, :], Act.Copy,
                            accum_out=scores[:, b, h * HK + j : h * HK + j + 1],
                        )
                if u % 4 == 3:
                    _epi_pair((u - 3) // 2)

    _legalize_sync_waits(nc)
    return nc


_NC_CACHE = None


def kernel(query, target, mask, Wq, bq, Wk, bk):
    global _NC_CACHE
    _install_axon_profile_shim()
    from concourse.bass_utils import run_bass_kernel_spmd

    query = np.ascontiguousarray(np.asarray(query, dtype=np.float32))
    target = np.ascontiguousarray(np.asarray(target, dtype=np.float32))
    mask = np.ascontiguousarray(np.asarray(mask, dtype=np.int32))
    Wq = np.ascontiguousarray(np.asarray(Wq, dtype=np.float32))
    bq = np.ascontiguousarray(np.asarray(bq, dtype=np.float32))
    Wk = np.ascontiguousarray(np.asarray(Wk, dtype=np.float32))
    bk = np.ascontiguousarray(np.asarray(bk, dtype=np.float32))

    if _NC_CACHE is None:
        _NC_CACHE = build_kernel()
    nc = _NC_CACHE

    in_maps = make_in_maps(query, target, mask, Wq, bq, Wk, bk)

    res = run_bass_kernel_spmd(nc, in_maps, list(range(NCORES)))
    outs = []
    for i in range(NCORES):
        aP = np.asarray(res.results[i]["alphaP"])  # [BS*CPB, 128]
        # undo the s = 1024h + 8p + j permutation
        a = aP.reshape(BS, 2, HK, 128).transpose(0, 1, 3, 2).reshape(BS, S)
        outs.append(a)
    return np.concatenate(outs, axis=0).astype(np.float32)


def make_in_maps(query, target, mask, Wq, bq, Wk, bk):
    WqT = np.ascontiguousarray(Wq.T)
    bqT = np.ascontiguousarray(bq.reshape(EC, 128).T)
    bkT = np.ascontiguousarray(bk.reshape(EC, 128).T)
    in_maps = []
    for i in range(NCORES):
        sl = slice(i * BS, (i + 1) * BS)
        m01 = (mask[sl] == 0).astype(np.float32)  # 1.0 keep / 0.0 masked
        m01P = np.ascontiguousarray(
            m01.reshape(BS, 2, 128, HK).transpose(2, 0, 1, 3).reshape(128, BS * CPB)
        )
        in_maps.append(
            {
                "queryT": np.ascontiguousarray(query[sl].T),
                "target": target[sl],
                "m01P": m01P,
                "WqT": WqT,
                "Wk": Wk,
                "bqT": bqT,
                "bkT": bkT,
            }
        )
    return in_maps


# revision 13
# speedup vs baseline: 1.1630x; 1.1630x over previous
"""Pointer-style attention kernel for Trainium2, SPMD over 8 NeuronCores.

Reference computation (per full batch B=128, S=2048, E=H=512):
    q  = query @ Wq.T + bq                    [B, H]
    k  = target @ Wk.T + bk                   [B, S, H]
    qk = einsum('bh,bsh->bs', q, k)           [B, S]
    qk = 10 * tanh(qk);  qk[mask==1] = -inf
    alpha = softmax(qk, axis=-1)

Key algebraic reformulation (exact in exact arithmetic):
    qk[b,s] = target[b,s,:] . qp[b,:] + qb[b]
      qp = (query @ Wq.T + bq) @ Wk           [B, E]
      qb = (query @ Wq.T + bq) . bk           [B]
This collapses the S*E*H einsum (137 GFLOP) into an S*E dot-product
stream (0.27 GFLOP), making the kernel HBM-bound on streaming `target`
(64 MiB per core; ~187 us floor at the 358 GB/s per-core HBM limit).

Distribution: data-parallel over batch; 16 batches per core, weights
replicated, no cross-core communication (softmax is per-row).

Per-core plan (v2):
  - target is streamed as 32 half-batch units of [128, 8, 512] fp32,
    alternating between the two HWDGE rings (sync/SP and scalar/ACT)
    so per-DMA completion latency on one ring hides under the other's
    stream. The s<->(partition,row) mapping s = 1024h + 8p + j makes
    each partition's 16 KB contiguous in HBM (fat descriptors); the
    resulting output permutation is undone on the host.
  - a single fused DVE tensor_tensor_reduce per (batch, s-row) does
    mul+reduce+bias in one pass: scores = qb + sum_e target*qp, with
    the product written to a stride-0 dummy (never materialized).
    ~4.5 us/unit on Vector vs 5.86 us/unit of DMA -> compute never
    backpressures the stream.
  - TensorE builds qp per batch as a broadcast matmul into PSUM
    (consumed directly by the DVE as in1) and handles all transposes;
    epilogue (tanh/exp/mask/softmax-normalize) runs per 2 batches in
    engine-idle slots, so the post-stream tail is only the last pair.
"""

import sys
import types

import numpy as np

B, S, E, H = 128, 2048, 512, 512
C_CLIP = 10.0
NCORES = 8
BS = B // NCORES  # 16 batches per core
EC = E // 128  # 4 e/h-chunks of 128
HK = 8  # s-rows per partition per unit; s = 1024h + 8p + j
CPB = 16  # score columns per batch (c = 8h + j)
NU = BS * 2  # 32 half-batch pipeline units


def _install_axon_profile_shim():
    """Make run_bass_kernel_spmd(trace=True) usable in this container:
    provide antenv.axon_hooks (NTFF profile hook via ctypes into the
    axon PJRT .so) and stub the S3 artifact upload."""
    try:
        if "antenv.axon_hooks" not in sys.modules:
            import antenv
            from trn_agent_boot.trn_boot import _ntff_profile_via_ctypes

            hook = _ntff_profile_via_ctypes("/opt/axon/libaxon_pjrt.so")
            mod = types.ModuleType("antenv.axon_hooks")
            mod._hook = hook
            mod.get_axon_ntff_profile_hook = lambda: mod._hook

            def _set(h):
                mod._hook = h

            mod.set_axon_ntff_profile_hook = _set
            sys.modules["antenv.axon_hooks"] = mod
            antenv.axon_hooks = mod
    except Exception:
        pass
    try:
        import concourse.bass_utils as bu

        bu.upload_artifacts = lambda tmpdir: str(tmpdir)
    except Exception:
        pass


def _legalize_sync_waits(nc):
    """This walrus build rejects instructions carrying more than a couple
    of sync-wait commands. After Tile scheduling, split each instruction's
    excess waits onto same-engine NOPs inserted immediately before it —
    sequencers execute in order, so semantics are identical."""
    import bass_rust
    from concourse import mybir

    n_split = 0
    for f in nc.m.functions:
        for blk in f.blocks:
            il = blk.instructions
            out = []
            changed = False
            for inst in il:
                si = inst.sync_info
                waits = list(si.on_wait) if si is not None else []
                cap = 2 if isinstance(inst, mybir.InstEventSemaphore) else 1
                if len(waits) > cap:
                    rest = waits[: len(waits) - cap]
                    for j, w in enumerate(rest):
                        nop = mybir.InstNoOp(
                            name=f"{inst.name}-swait{j}",
                            engine=inst.engine,
                            bass_nofuse=True,
                            sync_info=bass_rust.SyncInfo(on_wait=[w], on_update=[]),
                        )
                        out.append(nop)
                        n_split += 1
                    si.on_wait = waits[len(waits) - cap :]
                    inst.sync_info = si
                    changed = True
                out.append(inst)
            if changed:
                blk.instructions = out
    return n_split


def build_kernel():
    import concourse.bass as bass
    import concourse.tile as tile
    from concourse import mybir
    from concourse.masks import make_identity

    f32 = mybir.dt.float32
    bf16 = mybir.dt.bfloat16
    Alu = mybir.AluOpType
    Act = mybir.ActivationFunctionType

    nc = bass.Bass()
    # host passes layout-transformed views: queryT/WqT transposed, biases
    # in [p, chunk] column form, mask as a permuted keep-multiplier m01P
    queryT_d = nc.dram_tensor("queryT", [E, BS], f32, kind="ExternalInput")
    target_d = nc.dram_tensor("target", [BS, S, E], f32, kind="ExternalInput")
    m01P_d = nc.dram_tensor("m01P", [128, BS * CPB], f32, kind="ExternalInput")
    wqT_d = nc.dram_tensor("WqT", [E, H], f32, kind="ExternalInput")
    wk_d = nc.dram_tensor("Wk", [H, E], f32, kind="ExternalInput")
    bqT_d = nc.dram_tensor("bqT", [128, EC], f32, kind="ExternalInput")
    bkT_d = nc.dram_tensor("bkT", [128, EC], f32, kind="ExternalInput")
    alphaP_d = nc.dram_tensor("alphaP", [BS * CPB, 128], f32, kind="ExternalOutput")

    # unit (b, h): partition p holds s-rows 1024h + 8p + j, j=0..7 —
    # 16 KB contiguous per partition per unit
    units = target_d.rearrange("b (h p k) e -> (b h) p k e", h=2, p=128, k=HK)

    with tile.TileContext(nc) as tc:
        with (
            tc.tile_pool(name="singles", bufs=1) as singles,
            tc.tile_pool(name="tgt", bufs=8) as tgtp,
            tc.tile_pool(name="prod", bufs=2) as prodp,
            tc.tile_pool(name="trash", bufs=2) as trashp,
            tc.tile_pool(name="epi", bufs=2) as epip,
            tc.tile_pool(name="ppre", bufs=2, space="PSUM") as ppre,
            tc.tile_pool(name="pqpb", bufs=3, space="PSUM") as pqpb,
            tc.tile_pool(name="pepi", bufs=2, space="PSUM") as pepi,
        ):
            # ---- weight/query/mask DMAs: scalar(ACT) ring; wk on sync(SP)
            # ring ahead of the even target units. Target units alternate
            # rings, so both rings stream back-to-back from t=0.
            qT_sb = singles.tile([128, EC, BS], f32)  # queryT [e'-part, b]
            nc.scalar.dma_start(
                out=qT_sb, in_=queryT_d.rearrange("(m p) b -> p m b", p=128)
            )
            wqT_sb = singles.tile([128, EC, H], f32)  # [p, e'-chunk, h]
            for c in range(EC):
                nc.scalar.dma_start(
                    out=wqT_sb[:, c, :], in_=wqT_d[c * 128 : (c + 1) * 128, :]
                )
            bqT = singles.tile([128, EC], f32)
            bkT = singles.tile([128, EC], f32)
            nc.scalar.dma_start(out=bqT, in_=bqT_d[:, :])
            nc.scalar.dma_start(out=bkT, in_=bkT_d[:, :])
            wk_sb = singles.tile([128, EC, E], f32)  # [p, h-chunk, e]
            for c in range(EC):
                nc.sync.dma_start(
                    out=wk_sb[:, c, :], in_=wk_d[c * 128 : (c + 1) * 128, :]
                )
            m01T = singles.tile([128, BS, CPB], f32)  # keep-multiplier
            nc.scalar.dma_start(
                out=m01T, in_=m01P_d.rearrange("p (b c) -> p b c", b=BS)
            )

            ident = singles.tile([128, 128], f32)
            make_identity(nc, ident)
            ones_row = singles.tile([1, 128], f32)  # lhsT for partition-bcast
            nc.vector.memset(ones_row, 1.0)
            ones_col = singles.tile([128, 1], f32)  # lhsT for partition-sum
            nc.vector.memset(ones_col, 1.0)

            # Warm the PE clock gate (HAM) while the weight DMAs land.
            for _ in range(16):
                pwrm = ppre.tile([128, 128], f32, tag="pre")
                nc.tensor.matmul(pwrm, ident, ident, start=True, stop=True)

            # q = query @ Wq.T  [BS, H]
            q_sb = singles.tile([BS, H], f32)
            pq2 = ppre.tile([BS, H], f32, tag="pre")
            for m in range(EC):
                nc.tensor.matmul(
                    pq2, qT_sb[:, m, :], wqT_sb[:, m, :],
                    start=(m == 0), stop=(m == EC - 1),
                )
            nc.scalar.copy(q_sb, pq2)
            # qhT = (q + bq) transposed to [h-part, b]
            qhT = singles.tile([128, EC, BS], f32)
            for c in range(EC):
                pq3 = ppre.tile([128, BS], f32, tag="pre")
                nc.tensor.transpose(
                    pq3, q_sb[:, c * 128 : (c + 1) * 128], ident[0:BS, 0:BS]
                )
                nc.scalar.copy(qhT[:, c, :], pq3)
                nc.vector.tensor_scalar(
                    out=qhT[:, c, :], in0=qhT[:, c, :],
                    scalar1=bqT[:, c : c + 1], scalar2=None, op0=Alu.add,
                )

            # qb[b] = (q + bq) . bk, broadcast to qbb [128, BS] — feeds the
            # fused reduce as its per-partition init value.
            pqb = ppre.tile([BS, 1], f32, tag="pre")
            for c in range(EC):
                nc.tensor.matmul(
                    pqb, qhT[:, c, :], bkT[:, c : c + 1],
                    start=(c == 0), stop=(c == EC - 1),
                )
            qb_sb = singles.tile([BS, 1], f32)
            nc.scalar.copy(qb_sb, pqb)
            pqbT = ppre.tile([1, BS], f32, tag="pre")
            nc.tensor.transpose(pqbT, qb_sb, ident[0:BS, 0:BS])
            qbrow = singles.tile([1, BS], f32)
            nc.scalar.copy(qbrow, pqbT)
            pqbb = ppre.tile([128, BS], f32, tag="pre")
            nc.tensor.matmul(pqbb, ones_row, qbrow, start=True, stop=True)
            qbb = singles.tile([128, BS], f32)
            nc.scalar.copy(qbb, pqbb)

            scores = singles.tile([128, BS, CPB], f32)
            e2 = singles.tile([128, BS, CPB], f32)
            a_sb = singles.tile([128, BS, CPB], f32)
            part = singles.tile([128, BS], f32)
            dummy = singles.tile([128, 1], f32)  # stride-0 sink for TTR out

            def _epi_pair(b0):
                """tanh/exp/mask/normalize batches b0, b0+1 and DMA them out.
                The qb[b] bias lands here, folded into the tanh's bias."""
                for b in (b0, b0 + 1):
                    t_t = epip.tile([128, CPB], f32, tag="tanh")
                    nc.scalar.activation(
                        t_t, scores[:, b, :], Act.Tanh,
                        bias=qbb[:, b : b + 1], scale=1.0,
                    )
                    nc.scalar.activation(e2[:, b, :], t_t, Act.Exp, scale=C_CLIP)
                for b in (b0, b0 + 1):
                    # fused: e2 *= m01 (mask) and part = row-sum, one pass
                    nc.vector.scalar_tensor_tensor(
                        out=e2[:, b, :], in0=e2[:, b, :], scalar=0.0,
                        in1=m01T[:, b, :], op0=Alu.bypass, op1=Alu.mult,
                        accum_out=part[:, b : b + 1],
                    )
                pden = pepi.tile([1, 2], f32, tag="epi")
                nc.tensor.matmul(
                    pden, ones_col, part[:, b0 : b0 + 2], start=True, stop=True
                )
                recip = epip.tile([1, 2], f32, tag="recip")
                nc.vector.reciprocal(recip, pden)
                prb = pepi.tile([128, 2], f32, tag="epi")
                nc.tensor.matmul(prb, ones_row, recip, start=True, stop=True)
                rb = epip.tile([128, 2], f32, tag="rb")
                nc.scalar.copy(rb, prb)
                for b in (b0, b0 + 1):
                    nc.vector.tensor_scalar(
                        out=a_sb[:, b, :], in0=e2[:, b, :],
                        scalar1=rb[:, b - b0 : b - b0 + 1], scalar2=None,
                        op0=Alu.mult,
                    )
                pat = pepi.tile([32, 128], f32, tag="epi")
                nc.tensor.transpose(pat, a_sb[:, b0 : b0 + 2, :], ident)
                at = epip.tile([32, 128], f32, tag="at")
                nc.scalar.copy(at, pat)
                nc.sync.dma_start(
                    out=alphaP_d[b0 * CPB : (b0 + 2) * CPB, :], in_=at
                )

            # ---- main pipeline: stream target; per-unit reduction takes one
            # of two engine paths, balanced so both stay under the DMA rate:
            #   V path: fused scalar_tensor_tensor (V ~5.4 us/unit, S 0)
            #   S path: V multiply + ScalarE activation-accum (V ~4.2, S ~6.2)
            # The last two units go on the V path so the tail doesn't wait on
            # Scalar's longer chain.
            v_path = {0, 4, 8, 12, 16, 20, 24, 28, 30, 31}
            pb_cur = None
            for u in range(NU):
                b, h = divmod(u, 2)
                if h == 0:
                    # qp[b] broadcast across partitions, direct into PSUM
                    pb_cur = pqpb.tile([128, E], f32, tag="qpb")
                    for c in range(EC):
                        qrep = bass.AP(
                            tensor=qhT.tensor,
                            offset=qhT[:, c, b : b + 1].offset,
                            ap=[qhT.ap[0], [0, 128]],
                        )
                        nc.tensor.matmul(
                            pb_cur, qrep, wk_sb[:, c, :],
                            start=(c == 0), stop=(c == EC - 1),
                        )
                tgt = tgtp.tile([128, HK, E], f32, tag="tgt")
                nc.sync.dma_start(out=tgt, in_=units[u])
                if u in v_path:
                    for j in range(HK):
                        nc.vector.scalar_tensor_tensor(
                            out=dummy.broadcast_to((128, E)),
                            in0=tgt[:, j, :],
                            scalar=0.0,
                            in1=pb_cur,
                            op0=Alu.bypass,
                            op1=Alu.mult,
                            accum_out=scores[:, b, h * HK + j : h * HK + j + 1],
                        )
                else:
                    prod = prodp.tile([128, HK, E], f32, tag="prod")
                    pb_b = bass.AP(
                        tensor=pb_cur.tensor, offset=pb_cur.offset,
                        ap=[pb_cur.ap[0], [0, HK], pb_cur.ap[1]],
                    )
                    nc.vector.tensor_mul(prod, tgt, pb_b)
                    for j in range(HK):
                        tr = trashp.tile([128, E], bf16, tag="tr")
                        nc.scalar.activation(
                            tr, prod[:, j, :], Act.Copy,
                            accum_out=scores[:, b, h * HK + j : h * HK + j + 1],
                        )
                if u % 4 == 3:
                    _epi_pair((u - 3) // 2)

    _legalize_sync_waits(nc)
    return nc


_NC_CACHE = None


def kernel(query, target, mask, Wq, bq, Wk, bk):
    global _NC_CACHE
    _install_axon_profile_shim()
    from concourse.bass_utils import run_bass_kernel_spmd

    query = np.ascontiguousarray(np.asarray(query, dtype=np.float32))
    target = np.ascontiguousarray(np.asarray(target, dtype=np.float32))
    mask = np.ascontiguousarray(np.asarray(mask, dtype=np.int32))
    Wq = np.ascontiguousarray(np.asarray(Wq, dtype=np.float32))
    bq = np.ascontiguousarray(np.asarray(bq, dtype=np.float32))
    Wk = np.ascontiguousarray(np.asarray(Wk, dtype=np.float32))
    bk = np.ascontiguousarray(np.asarray(bk, dtype=np.float32))

    if _NC_CACHE is None:
        _NC_CACHE = build_kernel()
    nc = _NC_CACHE

    in_maps = make_in_maps(query, target, mask, Wq, bq, Wk, bk)

    res = run_bass_kernel_spmd(nc, in_maps, list(range(NCORES)))
    outs = []
    for i in range(NCORES):
        aP = np.asarray(res.results[i]["alphaP"])  # [BS*CPB, 128]
        # undo the s = 1024h + 8p + j permutation
        a = aP.reshape(BS, 2, HK, 128).transpose(0, 1, 3, 2).reshape(BS, S)
        outs.append(a)
    return np.concatenate(outs, axis=0).astype(np.float32)


def make_in_maps(query, target, mask, Wq, bq, Wk, bk):
    WqT = np.ascontiguousarray(Wq.T)
    bqT = np.ascontiguousarray(bq.reshape(EC, 128).T)
    bkT = np.ascontiguousarray(bk.reshape(EC, 128).T)
    in_maps = []
    for i in range(NCORES):
        sl = slice(i * BS, (i + 1) * BS)
        m01 = (mask[sl] == 0).astype(np.float32)  # 1.0 keep / 0.0 masked
        m01P = np.ascontiguousarray(
            m01.reshape(BS, 2, 128, HK).transpose(2, 0, 1, 3).reshape(128, BS * CPB)
        )
        in_maps.append(
            {
                "queryT": np.ascontiguousarray(query[sl].T),
                "target": target[sl],
                "m01P": m01P,
                "WqT": WqT,
                "Wk": Wk,
                "bqT": bqT,
                "bkT": bkT,
            }
        )
    return in_maps
